# revision 1
# baseline (speedup 1.0000x reference)
"""DNDT (deep neural decision tree) forward kernel for 8 Trainium2 NeuronCores.

Math (per batch row b of 16384):
  h[f,j]   = (x[b,f] * W[j] + bias[f,j]) / t,  W = [1..4], bias = cumsum([0,-sorted_cuts])
  bins     = softmax_j(h)                       # [6, 4]
  leaf     = kron(bins[0], ..., bins[5])        # [4096]
  out[b]   = leaf @ leaf_score                  # [10]

Device algorithm (pure data parallel, 2048 rows/core, batch-major layout
[128 partitions x 16 rows-per-partition x ...]):
  * softmax shift uses the analytic bound g(x) = (x + 3*relu(x))/t instead of a
    max-reduce (softmax is shift invariant; |h - g| <= 30 so exp can't overflow),
    folded as  h' = x*(W[j]-1)/t + bias[f,j]/t - 3/t*relu(x).
  * unnormalized bins E = exp(h'); normalizer applied once at the end via
    1/prod_f(sum_j E[f,j]).
  * leaf is never materialized: leaf = p0123 (x) p45 with p0123 = bins0*bins1*
    bins2*bins3 kron (256), p45 = bins4*bins5 kron (16).  Then
      out[b,c] = sum_v p45[b,v] * C[b,c,v],   C = p0123 @ S2,
    where S2[u, c*16+v] = leaf_score[u*16+v, c] - a [.,256]@[256,160] matmul
    instead of [.,4096]@[4096,10] (16x fewer PE MACs + 26x less DVE build).
  * p0123 is transposed 128x128 at a time through the PE (matmul needs the
    contraction dim on partitions); the whole matmul path runs in float32r
    (fp32 bits, fast PE streaming, ~1e-4 rounding - measured on HW; fp32
    matmuls cost 2 half-rate passes, ~4x slower).
  * a burst of junk matmuls on the x tile, emitted before the front, warms the
    PE HAM clock gate (1.2 -> 2.4 GHz) while the DVE builds the kron products;
    without it every matmul runs at half clock.
  * per-ISA limits, Matmult carries one sync-wait slot, so PE-feeding tensors
    are produced on a single engine each (Bacc's event-semaphore legalization
    handles the rest); constants are pre-broadcast on the host because a
    stride-0 broadcast DMA costs a ~3.3us SWDGE drain.
"""

import numpy as np

import concourse.bass as bass
import concourse.tile as tile
from concourse import bacc, mybir
from concourse.bass_utils import run_bass_kernel_spmd

N_CORES = 8
B = 16384
BC = B // N_CORES          # rows per core = 2048
P = 128                    # partitions
M = BC // P                # rows per partition = 16
NCHUNK = 2                 # pipeline chunks
CHM = M // NCHUNK          # rows per partition per chunk = 8
F32 = mybir.dt.float32
F32R = mybir.dt.float32r
N_WARM = 12                # junk matmuls to warm the PE clock gate


def _build_nc(neg3invt):
    nc = bacc.Bacc("TRN2", target_bir_lowering=False, debug=False,
                   num_devices=N_CORES)
    xd = nc.dram_tensor("x", [P, M * 6], F32, kind="ExternalInput")
    cstd = nc.dram_tensor("cst", [P, 3 * 24], F32, kind="ExternalInput")
    s2d = nc.dram_tensor("s2", [256, 160], F32R, kind="ExternalInput")
    idd = nc.dram_tensor("ident", [P, P], F32R, kind="ExternalInput")
    od = nc.dram_tensor("o", [P, M * 10], F32, kind="ExternalOutput")

    with tile.TileContext(nc) as tc:
        with tc.tile_pool(name="consts", bufs=1) as consts, \
             tc.tile_pool(name="work", bufs=2) as work, \
             tc.tile_pool(name="atp", bufs=8) as atp, \
             tc.tile_pool(name="ps_t", bufs=4, space="PSUM") as ps_t, \
             tc.tile_pool(name="ps_c", bufs=4, space="PSUM") as ps_c:
            # Parallel const loads on separate DMA queues; x/cst flow through
            # one ACT copy so downstream waits collapse onto the ACT sem.
            x_st = consts.tile([P, M, 6], F32)
            nc.sync.dma_start(out=x_st[:], in_=xd[:].rearrange("p (i f) -> p i f", i=M))
            # HAM warm-up: junk matmuls on the otherwise idle PE while the
            # front (DMAs, DVE H/E/p01/p23/A) runs.  fp32 (slow path) on
            # purpose: more PE-busy cycles per instruction.
            xw = x_st[:].rearrange("p i f -> p (i f)")

            def warm_mm(n):
                for _ in range(n):
                    wps = ps_t.tile([P, 2, P], F32R, tag="tp")
                    nc.tensor.matmul(wps[0:M * 6, 0, :M * 6].bitcast(mybir.dt.float32),
                                     lhsT=xw, rhs=xw[:, 0:M * 6], start=True, stop=True)
            warm_mm(21)
            cst_st = consts.tile([P, 3, 6, 4], F32)
            nc.sync.dma_start(out=cst_st[:].rearrange("p k f j -> p (k f j)"),
                              in_=cstd[:])
            s2_sb = consts.tile([P, 2, 160], F32R)
            nc.sync.dma_start(out=s2_sb[:], in_=s2d[:].rearrange("(k p) n -> p k n", p=P))
            ident = consts.tile([P, P], F32R)
            nc.sync.dma_start(out=ident[:], in_=idd[:])

            for c in range(NCHUNK):
                xv = x_st[:, c * CHM:(c + 1) * CHM, :]
                # r2 = -3/t * relu(x)   (fused max+mul)
                r2 = work.tile([P, CHM, 6, 1], F32, tag="r2")
                nc.vector.tensor_scalar(out=r2[:, :, :, 0], in0=xv,
                                        scalar1=0.0, scalar2=neg3invt,
                                        op0=mybir.AluOpType.max, op1=mybir.AluOpType.mult)
                H = work.tile([P, CHM, 6, 4], F32, tag="H")
                nc.vector.tensor_mul(H[:], xv[:, :, :, None].broadcast_to((P, CHM, 6, 4)),
                                     cst_st[:, 0:1, :, :].broadcast_to((P, CHM, 6, 4)))
                nc.vector.tensor_add(H[:], H[:], cst_st[:, 1:2, :, :].broadcast_to((P, CHM, 6, 4)))
                nc.vector.tensor_add(H[:], H[:], r2[:].broadcast_to((P, CHM, 6, 4)))
                E = work.tile([P, CHM, 6, 4], F32, tag="E")
                nc.scalar.activation(E[:].rearrange("p i f j -> p (i f j)"),
                                     H[:].rearrange("p i f j -> p (i f j)"),
                                     mybir.ActivationFunctionType.Exp)

                p01 = work.tile([P, CHM, 16], F32, tag="p01")
                p23 = work.tile([P, CHM, 16], F32, tag="p23")
                p45 = work.tile([P, CHM, 16], F32, tag="p45")
                for (pt, fa, fb) in ((p01, 0, 1), (p23, 2, 3)):
                    nc.vector.tensor_mul(
                        pt[:].rearrange("p i (a b) -> p i a b", a=4),
                        E[:, :, fa, :, None].broadcast_to((P, CHM, 4, 4)),
                        E[:, :, fb, None, :].broadcast_to((P, CHM, 4, 4)))
                A = work.tile([P, CHM, 256], F32R, tag="A")
                half = CHM // 2
                q = CHM // 4
                for hh in range(4):
                    sl = slice(hh * q, (hh + 1) * q)
                    nc.vector.tensor_mul(
                        A[:, sl, :].rearrange("p i (a b) -> p i a b", a=16),
                        p01[:, sl, :, None].broadcast_to((P, q, 16, 16)),
                        p23[:, sl, None, :].broadcast_to((P, q, 16, 16)))
                nc.vector.tensor_mul(
                    p45[:].rearrange("p i (a b) -> p i a b", a=4),
                    E[:, :, 4, :, None].broadcast_to((P, CHM, 4, 4)),
                    E[:, :, 5, None, :].broadcast_to((P, CHM, 4, 4)))
                Z = work.tile([P, CHM, 6], F32, tag="Z")
                nc.vector.tensor_reduce(Z[:], E[:], axis=mybir.AxisListType.X,
                                        op=mybir.AluOpType.add)
                zp = work.tile([P, CHM], F32, tag="zp")
                nc.vector.tensor_reduce(zp[:], Z[:], axis=mybir.AxisListType.X,
                                        op=mybir.AluOpType.mult)
                zr = work.tile([P, CHM, 1], F32, tag="zr")
                nc.vector.reciprocal(zr[:, :, 0], zp[:])
                # fold the softmax normalizer into p45: p45n = p45 * (1/prod Z)
                p45n = work.tile([P, CHM, 1, 16], F32, tag="p45n")
                nc.vector.tensor_mul(p45n[:, :, 0, :], p45[:],
                                     zr[:].broadcast_to((P, CHM, 16)))

                D = work.tile([P, CHM, 10, 16], F32, tag="D")
                O = work.tile([P, CHM, 10], F32, tag="O")
                for pair in range(CHM // 2):
                    cpp = ps_c.tile([P, 2, 160], F32, tag="cp")
                    for hhalf in range(2):
                        i = pair * 2 + hhalf
                        tp = ps_t.tile([P, 2, P], F32R, tag="tp")
                        for k in range(2):
                            nc.tensor.transpose(tp[:, k, :], A[:, i, k * P:(k + 1) * P],
                                                ident[:])
                        at2 = atp.tile([P, 2, P], F32R, tag="at")
                        nc.scalar.copy(out=at2[:], in_=tp[:])
                        nc.tensor.matmul(cpp[:, hhalf, :], lhsT=at2[:, 0, :],
                                         rhs=s2_sb[:, 0, :], start=True, stop=False)
                        nc.tensor.matmul(cpp[:, hhalf, :], lhsT=at2[:, 1, :],
                                         rhs=s2_sb[:, 1, :], start=False, stop=True)
                    sl = slice(pair * 2, pair * 2 + 2)
                    nc.vector.tensor_mul(
                        D[:, sl],
                        cpp[:].rearrange("p i (c v) -> p i c v", c=10),
                        p45n[:, sl].broadcast_to((P, 2, 10, 16)))
                    nc.vector.tensor_reduce(O[:, sl], D[:, sl],
                                            axis=mybir.AxisListType.X,
                                            op=mybir.AluOpType.add)
                nc.sync.dma_start(
                    out=od[:].rearrange("p (i c) -> p i c", i=M)[:, c * CHM:(c + 1) * CHM, :],
                    in_=O[:])
    nc.compile()
    return nc


_CACHE = {}


def kernel(x, cuts, leaf_score, temperature):
    x = np.ascontiguousarray(np.asarray(x, dtype=np.float32))
    cuts = np.asarray(cuts, dtype=np.float32)
    leaf_score = np.asarray(leaf_score, dtype=np.float32)
    invt = 1.0 / float(np.asarray(temperature).reshape(-1)[0])

    # host-side parameter prep (tiny)
    sc = np.sort(cuts, axis=1)
    bias = np.cumsum(np.concatenate([np.zeros((6, 1), np.float32), -sc], axis=1,
                                    dtype=np.float32), axis=1)          # [6,4]
    W = np.arange(1.0, 5.0, dtype=np.float32)
    w2 = np.tile(((W - 1.0) * invt)[None, :], (6, 1))                    # [6,4]
    bt = bias * invt                                                     # [6,4]
    r3 = np.zeros((6, 4), np.float32)
    r3[0, 0] = -3.0 * invt
    cst = np.ascontiguousarray(np.broadcast_to(
        np.stack([w2, bt, r3]).reshape(1, 72), (P, 72)).astype(np.float32))
    s2 = np.ascontiguousarray(
        leaf_score.reshape(256, 16, 10).transpose(0, 2, 1).reshape(256, 160))
    ident = np.eye(P, dtype=np.float32)

    key = ("nc", float(invt))
    if key not in _CACHE:
        _CACHE[key] = _build_nc(-3.0 * invt)
        _CACHE["nc"] = _CACHE[key]
    nc = _CACHE[key]

    xs = x.reshape(N_CORES, P, M * 6)
    in_maps = [{"x": xs[i], "cst": cst, "s2": s2, "ident": ident}
               for i in range(N_CORES)]
    res = run_bass_kernel_spmd(nc, in_maps, list(range(N_CORES))).results
    out = np.concatenate([r["o"].reshape(BC, 10) for r in res], axis=0)
    return out.astype(np.float32)

